# revision 20
# baseline (speedup 1.0000x reference)
"""Paged-attention decode (GQA) on 8 Trainium2 NeuronCores.

Strategy
--------
The reference computes, per sequence b and kv-head h, attention of 4 query
heads over the first context_lens[b] tokens of a block-paged KV cache (with
the new token's k/v scattered in at slot_mapping[b] first).

Host side: gather each sequence's KV context from the paged cache (applying
the slot_mapping scatter on the gathered copy), then flatten ALL
(sequence, kv-head) work into a stream of 128-token tiles.  Tiles are
distributed evenly across the 8 cores (a (b,h) unit's tiles may span cores;
the final combine is a cheap host-side reduction).  Per tile the device
needs:
  kT   [128 d, 128 tok]   K transposed (zero-padded past the context)
  v    [128 tok, 128 d]   V (zero-padded)
  qT   [128 d, 4 g]       the unit's queries, replicated per tile
  mask [128 tok, 4 g]     1.0 for valid tokens, 0.0 for padding

Device kernel (identical SPMD program on all 8 cores), per group of GS
tiles:
  scoresT[tok, g] = kT.T @ qT                (PE, per tile)
  p = exp(SCALE * scoresT) * mask            (ACT exp + DVE mul, batched)
  outT[d, g]  = v.T @ p                      (PE, per tile)
  den[1, g]   = ones.T @ p                   (PE, per group)
Unnormalized per-tile results stream back to HBM; the host sums tiles of
each unit and divides by the denominator.  exp is taken without
max-subtraction (scores are ~N(0,1) here, so no overflow risk), which makes
the per-tile partials exactly summable.

Precision/speed modes (_KV_DT): fp32 LDWEIGHTS is ~4x slow on this target
(walrus ldw-opt disabled), so plain fp32 is PE-bound.  "bf16x2" splits each
fp32 value into bf16 hi + bf16 lo residual and uses 3 PE products
(hi*hi + hi*lo + lo*hi), recovering ~1e-5 accuracy at bf16 PE speed with
fp32-sized HBM traffic.  "bfloat16" is ~2x faster still (half traffic) at
~4e-3 accuracy.
"""

import numpy as np

_TS = 128        # tokens per tile (matmul output partition limit)
_GS = 32         # tiles per DMA/compute group
_NC = 8          # NeuronCores
_OC = 4          # groups per output-DMA chunk
_SCALE = 0.08838834764831845
_KV_DT = "bf16x2"   # float32 | float32r | bfloat16 | bf16x2


def _build_program(n_tiles, n_groups, reps=1):
    """One SPMD program; all per-core variation lives in the input data.

    reps>1 wraps the whole body in an on-device For_i loop that redoes the
    identical work -- used only for timing (slope vs reps isolates device
    time from host/relay dispatch overhead).
    """
    import contextlib

    import concourse.bacc as bacc
    import concourse.tile as tile
    import concourse.mybir as mybir

    f32 = mybir.dt.float32
    bf16 = mybir.dt.bfloat16
    split = _KV_DT == "bf16x2"
    r32 = _KV_DT == "float32r"
    mdt = bf16 if split else getattr(mybir.dt, _KV_DT)
    # float32r tiles may only be written by DMA / rounding copies; ACT, DVE
    # and memset work on plain fp32 (with a rounding copy before PE).
    edt = f32 if (r32 or split) else mdt
    Exp = mybir.ActivationFunctionType.Exp
    D = 128

    nc = bacc.Bacc("TRN2", target_bir_lowering=False, debug=False, num_devices=_NC)
    n_str = 2 if split else 1
    kT = [
        nc.dram_tensor(f"kT{i}", [128, n_tiles * _TS], mdt, kind="ExternalInput")
        for i in range(n_str)
    ]
    vg = [
        nc.dram_tensor(f"vg{i}", [n_groups, 128, _GS * D], mdt, kind="ExternalInput")
        for i in range(n_str)
    ]
    qT = [
        nc.dram_tensor(f"qT{i}", [128, n_tiles * 4], mdt, kind="ExternalInput")
        for i in range(n_str)
    ]
    mk = nc.dram_tensor("mk", [128, n_tiles * 4], edt, kind="ExternalInput")
    outT = nc.dram_tensor("outT", [128, n_tiles * 4], f32, kind="ExternalOutput")
    den = nc.dram_tensor("den", [1, n_tiles * 4], f32, kind="ExternalOutput")

    with tile.TileContext(nc) as tc:
        with contextlib.ExitStack() as ctx:
            singles = ctx.enter_context(tc.tile_pool(name="singles", bufs=1))
            kpool = ctx.enter_context(tc.tile_pool(name="kpool", bufs=4))
            vpool = ctx.enter_context(tc.tile_pool(name="vpool", bufs=4))
            ptpool = ctx.enter_context(tc.tile_pool(name="ptpool", bufs=3))
            otpool = ctx.enter_context(tc.tile_pool(name="otpool", bufs=2))
            dnpool = ctx.enter_context(tc.tile_pool(name="dnpool", bufs=2))
            pspool = ctx.enter_context(
                tc.tile_pool(name="pspool", bufs=3, space="PSUM")
            )
            popool = ctx.enter_context(
                tc.tile_pool(name="popool", bufs=3, space="PSUM")
            )
            pdpool = ctx.enter_context(
                tc.tile_pool(name="pdpool", bufs=2, space="PSUM")
            )

            ones = singles.tile([128, 1], mdt)
            if r32:
                ones_f = singles.tile([128, 1], f32)
                nc.vector.memset(ones_f, 1.0)
                nc.vector.tensor_copy(ones, ones_f)
            else:
                nc.vector.memset(ones, 1.0)
            qts = []
            for i in range(n_str):
                t = singles.tile([128, n_tiles * 4], mdt, tag=f"qts{i}")
                nc.sync.dma_start(out=t, in_=qT[i].ap())
                qts.append(t)
            mks = singles.tile([128, n_tiles * 4], edt)
            nc.sync.dma_start(out=mks, in_=mk.ap())

            def body():
              ot = dt = None
              for gi in range(n_groups):
                kts, vts = [], []
                for i in range(n_str):
                    kt = kpool.tile([128, _GS * _TS], mdt, tag=f"kt{i}")
                    nc.sync.dma_start(
                        out=kt,
                        in_=kT[i].ap()[:, gi * _GS * _TS : (gi + 1) * _GS * _TS],
                    )
                    kts.append(kt)
                    vt = vpool.tile([128, _GS * D], mdt, tag=f"vt{i}")
                    nc.sync.dma_start(out=vt, in_=vg[i].ap()[gi])
                    vts.append(vt)

                ps = pspool.tile([128, _GS * 4], f32)
                for j in range(_GS):
                    out_j = ps[:, j * 4 : (j + 1) * 4]
                    k_j = [kt[:, j * _TS : (j + 1) * _TS] for kt in kts]
                    q_j = [
                        t[:, (gi * _GS + j) * 4 : (gi * _GS + j + 1) * 4]
                        for t in qts
                    ]
                    if split:
                        nc.tensor.matmul(out_j, k_j[0], q_j[0], start=True, stop=False)
                        nc.tensor.matmul(out_j, k_j[0], q_j[1], start=False, stop=False)
                        nc.tensor.matmul(out_j, k_j[1], q_j[0], start=False, stop=True)
                    else:
                        nc.tensor.matmul(out_j, k_j[0], q_j[0], start=True, stop=True)

                pt = ptpool.tile([128, _GS * 4], edt)
                nc.scalar.activation(out=pt, in_=ps, func=Exp, scale=_SCALE)
                nc.vector.tensor_mul(
                    pt, pt, mks[:, gi * _GS * 4 : (gi + 1) * _GS * 4]
                )
                if split:
                    phi = ptpool.tile([128, _GS * 4], bf16, tag="phi")
                    nc.vector.tensor_copy(phi, pt)
                    plo = ptpool.tile([128, _GS * 4], bf16, tag="plo")
                    nc.vector.tensor_sub(plo, pt, phi)
                    pts = [phi, plo]
                elif r32:
                    pt_r = ptpool.tile([128, _GS * 4], mdt, tag="pt_r")
                    nc.vector.tensor_copy(pt_r, pt)
                    pts = [pt_r]
                else:
                    pts = [pt]

                po = popool.tile([128, _GS * 4], f32)
                for j in range(_GS):
                    out_j = po[:, j * 4 : (j + 1) * 4]
                    v_j = [vt[:, j * D : (j + 1) * D] for vt in vts]
                    p_j = [t[:, j * 4 : (j + 1) * 4] for t in pts]
                    if split:
                        nc.tensor.matmul(out_j, v_j[0], p_j[0], start=True, stop=False)
                        nc.tensor.matmul(out_j, v_j[0], p_j[1], start=False, stop=False)
                        nc.tensor.matmul(out_j, v_j[1], p_j[0], start=False, stop=True)
                    else:
                        nc.tensor.matmul(out_j, v_j[0], p_j[0], start=True, stop=True)

                pd = pdpool.tile([1, _GS * 4], f32)
                if split:
                    nc.tensor.matmul(pd, ones, pts[0], start=True, stop=False)
                    nc.tensor.matmul(pd, ones, pts[1], start=False, stop=True)
                else:
                    nc.tensor.matmul(pd, ones, pts[0], start=True, stop=True)

                ci = gi % _OC
                if ci == 0:
                    ot = otpool.tile([128, _OC * _GS * 4], f32)
                    dt = dnpool.tile([1, _OC * _GS * 4], f32)
                nc.vector.tensor_copy(ot[:, ci * _GS * 4 : (ci + 1) * _GS * 4], po)
                nc.vector.tensor_copy(dt[:, ci * _GS * 4 : (ci + 1) * _GS * 4], pd)
                if ci == _OC - 1 or gi == n_groups - 1:
                    base = (gi - ci) * _GS * 4
                    width = (ci + 1) * _GS * 4
                    nc.sync.dma_start(
                        out=outT.ap()[:, base : base + width], in_=ot[:, :width]
                    )
                    nc.sync.dma_start(
                        out=den.ap()[:, base : base + width], in_=dt[:, :width]
                    )

            if reps > 1:
                with tc.For_i(0, reps, 1):
                    body()
            else:
                body()
    nc.compile()
    return nc


def _split_hi_lo(a):
    import ml_dtypes

    hi = np.ascontiguousarray(a.astype(ml_dtypes.bfloat16))
    lo = np.ascontiguousarray((a - hi.astype(np.float32)).astype(ml_dtypes.bfloat16))
    return hi, lo


def _prepare(q, k, v, k_cache, v_cache, slot_mapping, block_tables, context_lens):
    """Host-side gather/pack.  Returns (n_tiles, n_groups, in_maps, meta)."""
    import ml_dtypes

    q = np.ascontiguousarray(np.asarray(q, dtype=np.float32))
    k = np.ascontiguousarray(np.asarray(k, dtype=np.float32))
    v = np.ascontiguousarray(np.asarray(v, dtype=np.float32))
    k_cache = np.asarray(k_cache)
    v_cache = np.asarray(v_cache)
    B, H, D = q.shape
    NB, BS, KVH, _ = k_cache.shape
    G = H // KVH
    MAX_S = block_tables.shape[1] * BS
    ctx = np.clip(np.asarray(context_lens, dtype=np.int64), 0, MAX_S)
    slot = np.asarray(slot_mapping, dtype=np.int64)
    bt = np.asarray(block_tables, dtype=np.int64)

    # slot_mapping scatter: later sequences overwrite earlier on duplicate
    # slots (matches sequential scatter semantics of the reference).
    patch = {}
    for b in range(B):
        patch[int(slot[b])] = b
    blk_patches = {}
    for s, pb in patch.items():
        blk_patches.setdefault(s // BS, []).append((s % BS, pb))

    # per-sequence gathered KV ([S, KVH, D]), scatter applied
    Ks, Vs = [None] * B, [None] * B
    for b in range(B):
        S = int(ctx[b])
        if S == 0:
            continue
        nblk = (S + BS - 1) // BS
        idx = bt[b, :nblk]
        Kb = k_cache[idx].reshape(nblk * BS, KVH, D)
        Vb = v_cache[idx].reshape(nblk * BS, KVH, D)
        for j, blkid in enumerate(idx):
            for off, pb in blk_patches.get(int(blkid), ()):
                pos = j * BS + off
                if pos < S:
                    Kb[pos] = k[pb]
                    Vb[pos] = v[pb]
        Ks[b], Vs[b] = Kb[:S], Vb[:S]

    # flat tile stream: (b, h, tok0, n_valid)
    tiles = []
    for b in range(B):
        S = int(ctx[b])
        for h in range(KVH):
            for t0 in range(0, S, _TS):
                tiles.append((b, h, t0, min(_TS, S - t0)))
    t_total = len(tiles)
    per_core = -(-t_total // _NC)
    n_tiles = -(-per_core // _GS) * _GS  # multiple of the group size
    n_groups = n_tiles // _GS

    split = _KV_DT == "bf16x2"
    npdt = (
        ml_dtypes.bfloat16 if _KV_DT == "bfloat16" else np.float32
    )
    mask_dt = np.float32 if _KV_DT in ("float32r", "float32", "bf16x2") else npdt

    in_maps = []
    core_tiles = []
    for c in range(_NC):
        ct = tiles[c * n_tiles : (c + 1) * n_tiles]
        core_tiles.append(ct)
        K_pack = np.zeros((n_tiles, _TS, D), np.float32)
        V_pack = np.zeros((n_tiles, _TS, D), np.float32)
        Q_pack = np.zeros((n_tiles, G, D), np.float32)
        M_pack = np.zeros((n_tiles, _TS), np.float32)
        for t, (b, h, t0, nv) in enumerate(ct):
            K_pack[t, :nv] = Ks[b][t0 : t0 + nv, h, :]
            V_pack[t, :nv] = Vs[b][t0 : t0 + nv, h, :]
            Q_pack[t] = q[b, h * G : (h + 1) * G, :]
            M_pack[t, :nv] = 1.0
        kT_all = K_pack.transpose(2, 0, 1).reshape(128, n_tiles * _TS)
        v_grp = (
            V_pack.reshape(n_groups, _GS, _TS, D)
            .transpose(0, 2, 1, 3)
            .reshape(n_groups, _TS, _GS * D)
        )
        qT_all = Q_pack.transpose(2, 0, 1).reshape(128, n_tiles * G)
        mask_all = (
            np.broadcast_to(M_pack.T[:, :, None], (_TS, n_tiles, G))
            .astype(mask_dt)
            .reshape(128, n_tiles * G)
        )
        m = {"mk": mask_all}
        if split:
            m["kT0"], m["kT1"] = _split_hi_lo(kT_all)
            m["vg0"], m["vg1"] = _split_hi_lo(v_grp)
            m["qT0"], m["qT1"] = _split_hi_lo(qT_all)
        else:
            m["kT0"] = np.ascontiguousarray(kT_all.astype(npdt))
            m["vg0"] = np.ascontiguousarray(v_grp.astype(npdt))
            m["qT0"] = np.ascontiguousarray(qT_all.astype(npdt))
        in_maps.append(m)

    meta = (B, H, KVH, G, D, core_tiles)
    return n_tiles, n_groups, in_maps, meta


def _finish(results, n_tiles, meta):
    B, H, KVH, G, D, core_tiles = meta
    num = np.zeros((B, KVH, D, G), np.float64)
    den = np.zeros((B, KVH, G), np.float64)
    for c in range(_NC):
        oT = results[c]["outT"].reshape(128, n_tiles, G).astype(np.float64)
        dn = results[c]["den"].reshape(n_tiles, G).astype(np.float64)
        for t, (b, h, t0, nv) in enumerate(core_tiles[c]):
            num[b, h] += oT[:, t, :]
            den[b, h] += dn[t]
    with np.errstate(invalid="ignore", divide="ignore"):
        o = num / den[:, :, None, :]
    return np.ascontiguousarray(o.transpose(0, 1, 3, 2)).reshape(B, H, D).astype(
        np.float32
    )


def kernel(q, k, v, k_cache, v_cache, slot_mapping, block_tables, context_lens):
    from concourse.bass_utils import run_bass_kernel_spmd

    n_tiles, n_groups, in_maps, meta = _prepare(
        q, k, v, k_cache, v_cache, slot_mapping, block_tables, context_lens
    )
    nc = _build_program(n_tiles, n_groups)
    res = run_bass_kernel_spmd(nc, in_maps, core_ids=list(range(_NC)), trace=False)
    return _finish(res.results, n_tiles, meta)
